# revision 15
# baseline (speedup 1.0000x reference)
"""Context-Query (BiDAF-style) attention kernel for Trainium2, 8 NeuronCores.

Problem (per batch b of 64):
  Ct = C[b].T (Lc,D), Qt = Q[b].T (Lq,D), w = [w1,w2,w3] each (D,)
  S  = Ct@w1 + (Qt@w2).T + (Ct*w3)@Qt.T                     (Lc,Lq)
  S1 = softmax_m(S), S2 = softmax_l(S)
  A  = S1@Qt, Bv = S1@(S2.T@Ct)      (associativity: avoids Lc x Lc matrix)
  out[b] = concat([Ct, A, Ct*A, Ct*Bv], axis=1).T           (4D, Lc)

Sharding: pure data-parallel, batch 64 -> 8 cores x 8 batches.

Device computes, per batch, in bf16 with f32 accumulation:
  E = exp(S^T + p2)        (m-part, l-free; p2[m] = Qt@w2 host-computed bias)
  r2raw[m] = sum_l E[m,l]  (accum during exp eviction)
  E_A = E^T via one DMA xbar-transpose instruction per m-chunk (l-part, m)
  R1[l] = sum_m E_A        (DVE free-axis reduces; shipped to host)
  A_un^T  = Qt^T @ E       (missing 1/R1[l]; host divides)
  T' = E_A^T x CbT in (m,d) layout; T = T'/r2raw (the e^{p2} factor cancels,
       so no second exp / second score pass is needed)
  Bv_un^T = T^T @ E        (missing 1/R1[l])
Host (numpy, f32): out = [C, A, C*A, C*Bv] with A = A_un/R1, Bv = Bv_un/R1.
Host also pre-packs per batch (all bf16): [Cb | Qb | QbT | CbT | p2-as-bf16x4]
so the only on-device transposes are the two E xbar transposes.
"""

import os
import threading

import numpy as np
import ml_dtypes

B, D, LC, LQ = 64, 128, 1024, 256
NCORES = 8
BPC = B // NCORES  # batches per core
BF16 = ml_dtypes.bfloat16

# packed input column offsets
OFF_CB = 0
OFF_QB = OFF_CB + LC          # 1024
OFF_QBT = OFF_QB + LQ         # 1280
OFF_CBT = OFF_QBT + LQ        # 1536
OFF_P2 = OFF_CBT + LC         # 2560
NCOLS = OFF_P2 + 4            # 2564

_lock = threading.Lock()
_cache: dict = {}


def _build_program():
    import concourse.bass as bass
    import concourse.bacc as bacc
    import concourse.mybir as mybir
    import concourse.tile as tile
    from contextlib import ExitStack

    f32 = mybir.dt.float32
    bf16 = mybir.dt.bfloat16
    MUL = mybir.AluOpType.mult
    ADD = mybir.AluOpType.add
    EXP = mybir.ActivationFunctionType.Exp
    X = mybir.AxisListType.X

    nc = bacc.Bacc("TRN2", target_bir_lowering=False)
    CQd = nc.declare_dram_parameter("CQ", [BPC, D, NCOLS], bf16, False)
    Wd = nc.declare_dram_parameter("w", [3 * D], f32, False)
    Od = nc.declare_dram_parameter("OUT", [BPC, D, 2, LC], bf16, True)
    R1d = nc.declare_dram_parameter("R1", [BPC, D, 8], f32, True)

    with ExitStack() as ctx:
        tc = ctx.enter_context(tile.TileContext(nc))
        const = ctx.enter_context(tc.tile_pool(name="const", bufs=1))
        # PSUM: big ring (128,1024)f32 tiles x3 bufs = 6 banks; tt ring 2 banks
        psb = ctx.enter_context(tc.tile_pool(name="psb", bufs=3, space="PSUM"))
        pst = ctx.enter_context(tc.tile_pool(name="pst", bufs=2, space="PSUM"))
        io = ctx.enter_context(tc.tile_pool(name="io", bufs=4))
        mid = ctx.enter_context(tc.tile_pool(name="mid", bufs=4))
        ep = ctx.enter_context(tc.tile_pool(name="ep", bufs=6))
        sm = ctx.enter_context(tc.tile_pool(name="sm", bufs=4))

        wt = const.tile([D, 3], f32)
        nc.sync.dma_start(wt[:], Wd.rearrange("(t d) -> d t", d=D))
        w1c, w3c = wt[:, 0:1], wt[:, 2:3]

        def emit_load(b):
            cq = io.tile([D, NCOLS], bf16, tag="cq", name=f"cq{b}")
            nc.sync.dma_start(cq[:], CQd[b])
            return cq

        def emit_scores(b, cq):
            """rhs1, score matmuls, exp (+row sums), E^T transposes."""
            cb = cq[:, OFF_CB : OFF_CB + LC]
            qb = cq[:, OFF_QB : OFF_QB + LQ]
            p2 = cq[:, OFF_P2 : OFF_P2 + 4].bitcast(f32)  # (128, 2) f32

            rhs1 = sm.tile([D, LQ], bf16, tag="rhs1", name=f"rhs1{b}")
            nc.gpsimd.tensor_scalar(rhs1[:], qb, w3c, w1c, op0=MUL, op1=ADD)

            r2raw = sm.tile([D, 2], f32, tag="r2raw", name=f"r2raw{b}")
            ea = mid.tile([D, 8, LQ], bf16, tag="ea", name=f"ea{b}")
            e1t = []
            for j in range(2):
                sb_ps = psb.tile([D, LC], f32, tag="psbig", name=f"sb{b}_{j}")
                lhs = rhs1[:, 128 * j : 128 * (j + 1)]
                for h in range(2):
                    nc.tensor.matmul(
                        sb_ps[:, 512 * h : 512 * (h + 1)], lhs,
                        cb[:, 512 * h : 512 * (h + 1)], start=True, stop=True,
                    )
                e = ep.tile([D, LC], bf16, tag="e1t", name=f"e{b}_{j}")
                nc.scalar.activation(
                    e[:], sb_ps[:], EXP, bias=p2[:, j : j + 1],
                    accum_out=r2raw[:, j : j + 1],
                )
                e1t.append(e)
                nc.sync.dma_start(
                    ea[:, :, 128 * j : 128 * (j + 1)], e[:], transpose=True
                )
            r2i = sm.tile([D, 2], f32, tag="r2i", name=f"r2i{b}")
            nc.vector.reciprocal(r2i[:], r2raw[:])
            return cq, ea, e1t, r2i

        def emit_tail(b, st):
            """A, T, Bv matmuls, R1 reduces, evictions, output DMAs."""
            cq, ea, e1t, r2i = st
            o1bv = io.tile([D, 2, LC], bf16, tag="o1bv", name=f"o1bv{b}")

            # A_un^T = Qt^T @ E  (d-part, l-free)
            a_ps = psb.tile([D, LC], f32, tag="psbig", name=f"a{b}")
            for j in range(2):
                for h in range(2):
                    nc.tensor.matmul(
                        a_ps[:, 512 * h : 512 * (h + 1)],
                        cq[:, OFF_QBT + 128 * j : OFF_QBT + 128 * (j + 1)],
                        e1t[j][:, 512 * h : 512 * (h + 1)],
                        start=(j == 0), stop=(j == 1),
                    )
            nc.scalar.copy(o1bv[:, 0, :], a_ps[:])

            # T'[m-chunk j] = sum_l E_A[l, m-chunk j] x CbT[l, :]
            tt_ps = []
            for j in range(2):
                tp = pst.tile([D, D], f32, tag="ttps", name=f"tt{b}_{j}")
                for p in range(8):
                    nc.tensor.matmul(
                        tp[:], ea[:, p, 128 * j : 128 * (j + 1)],
                        cq[:, OFF_CBT + 128 * p : OFF_CBT + 128 * (p + 1)],
                        start=(p == 0), stop=(p == 7),
                    )
                tt_ps.append(tp)

            # R1[l] = sum_m E_A[l, m]  (ready work while T matmuls run)
            r1h = sm.tile([D, 8, 2], f32, tag="r1h", name=f"r1h{b}")
            for j in range(2):
                nc.vector.tensor_reduce(
                    r1h[:, :, j], ea[:, :, 128 * j : 128 * (j + 1)], axis=X, op=ADD
                )
            r1small = sm.tile([D, 8], f32, tag="r1small", name=f"r1s{b}")
            nc.vector.tensor_tensor(
                r1small[:], r1h[:, :, 0], r1h[:, :, 1], op=ADD
            )

            # T = T'/r2raw, evicted straight from PSUM (m-part, d-free)
            tsb = sm.tile([D, LQ], bf16, tag="tsb", name=f"tsb{b}")
            for j in range(2):
                nc.vector.tensor_scalar(
                    tsb[:, 128 * j : 128 * (j + 1)], tt_ps[j][:],
                    r2i[:, j : j + 1], None, op0=MUL,
                )

            # Bv_un^T = T^T @ E  (d-part, l-free)
            bv_ps = psb.tile([D, LC], f32, tag="psbig", name=f"bv{b}")
            for j in range(2):
                for h in range(2):
                    nc.tensor.matmul(
                        bv_ps[:, 512 * h : 512 * (h + 1)],
                        tsb[:, 128 * j : 128 * (j + 1)],
                        e1t[j][:, 512 * h : 512 * (h + 1)],
                        start=(j == 0), stop=(j == 1),
                    )
            nc.vector.tensor_copy(o1bv[:, 1, :], bv_ps[:])

            nc.gpsimd.dma_start(Od[b], o1bv[:])
            nc.gpsimd.dma_start(R1d[b], r1small[:])

        # software-pipelined emission: per-engine queues are FIFO, so batch
        # k+1's (ready) scores work is queued ahead of batch k's transpose-
        # gated tail work, keeping PE/ACT/DVE streaming through DMA latency.
        cqs = {0: emit_load(0)}
        st = {0: emit_scores(0, cqs[0])}
        cqs[1] = emit_load(1)
        for k in range(BPC):
            if k + 1 < BPC:
                st[k + 1] = emit_scores(k + 1, cqs[k + 1])
            if k + 2 < BPC:
                cqs[k + 2] = emit_load(k + 2)
            emit_tail(k, st.pop(k))

    nc.compile()
    return nc


def _get_program():
    with _lock:
        if "nc" not in _cache:
            _cache["nc"] = _build_program()
        return _cache["nc"]


def _pack_inputs(C, Q, w):
    """Per batch: [Cb | Qb | QbT | CbT | p2-as-bf16x4] as (B, D, NCOLS) bf16."""
    Cb = C.astype(BF16)
    Qb = Q.astype(BF16)
    # [m%128, m//128, d] layout for QbT, [l%128, l//128, d] for CbT
    QbT = (
        Qb.transpose(0, 2, 1).reshape(B, 2, 128, D).transpose(0, 2, 1, 3)
        .reshape(B, 128, 2 * D)
    )
    CbT = (
        Cb.transpose(0, 2, 1).reshape(B, 8, 128, D).transpose(0, 2, 1, 3)
        .reshape(B, 128, 8 * D)
    )
    p2 = np.einsum("d,bdm->bm", w[D : 2 * D], Q)
    p2c = p2.reshape(B, 2, 128).transpose(0, 2, 1).astype(np.float32)  # (B,128,2)
    p2b = np.ascontiguousarray(p2c).view(BF16)  # (B, 128, 4)
    CQ = np.empty((B, D, NCOLS), dtype=BF16)
    CQ[:, :, OFF_CB:OFF_QB] = Cb
    CQ[:, :, OFF_QB:OFF_QBT] = Qb
    CQ[:, :, OFF_QBT:OFF_CBT] = QbT
    CQ[:, :, OFF_CBT:OFF_P2] = CbT
    CQ[:, :, OFF_P2:NCOLS] = p2b
    return CQ


def kernel(C, Q, cmask, qmask, w, **_):
    # cmask/qmask are identically 1.0 for this problem; softmax masking with
    # all-ones masks is the identity, so they do not enter the computation.
    from concourse.bass_utils import run_bass_kernel_spmd

    nc = _get_program()
    C = np.ascontiguousarray(np.asarray(C), dtype=np.float32)
    Q = np.ascontiguousarray(np.asarray(Q), dtype=np.float32)
    w = np.ascontiguousarray(np.asarray(w), dtype=np.float32)
    CQ = _pack_inputs(C, Q, w)
    in_maps = [
        {
            "CQ": np.ascontiguousarray(CQ[i * BPC : (i + 1) * BPC]),
            "w": w,
        }
        for i in range(NCORES)
    ]
    res = run_bass_kernel_spmd(
        nc, in_maps, core_ids=list(range(NCORES)),
        trace=bool(int(os.environ.get("KERNEL_TRACE", "0"))),
    )
    if os.environ.get("KERNEL_RESULT_STASH") is not None:
        _cache["last_result"] = res

    out = np.empty((B, 4 * D, LC), dtype=np.float32)
    out[:, 0:D] = C
    for i in range(NCORES):
        sl = slice(i * BPC, (i + 1) * BPC)
        ob = np.asarray(res.results[i]["OUT"])  # (BPC, D, 2, LC) bf16
        r1 = np.asarray(res.results[i]["R1"])   # (BPC, D, 8) f32
        r1full = r1.transpose(0, 2, 1).reshape(BPC, LC)  # l = 128*blk + p
        inv = (1.0 / r1full)[:, None, :]
        a = ob[:, :, 0, :].astype(np.float32) * inv
        bv = ob[:, :, 1, :].astype(np.float32) * inv
        out[sl, D : 2 * D] = a
        out[sl, 2 * D : 3 * D] = C[sl] * a
        out[sl, 3 * D : 4 * D] = C[sl] * bv
    return out


# revision 16
# speedup vs baseline: 1.0301x; 1.0301x over previous
"""Context-Query (BiDAF-style) attention kernel for Trainium2, 8 NeuronCores.

Problem (per batch b of 64):
  Ct = C[b].T (Lc,D), Qt = Q[b].T (Lq,D), w = [w1,w2,w3] each (D,)
  S  = Ct@w1 + (Qt@w2).T + (Ct*w3)@Qt.T                     (Lc,Lq)
  S1 = softmax_m(S), S2 = softmax_l(S)
  A  = S1@Qt, Bv = S1@(S2.T@Ct)      (associativity: avoids Lc x Lc matrix)
  out[b] = concat([Ct, A, Ct*A, Ct*Bv], axis=1).T           (4D, Lc)

Sharding: pure data-parallel, batch 64 -> 8 cores x 8 batches.

Device computes, per batch, in bf16 with f32 accumulation:
  E = exp(S^T + p2)        (m-part, l-free; p2[m] = Qt@w2 host-computed bias)
  r2raw[m] = sum_l E[m,l]  (accum during exp eviction)
  E_A = E^T via one DMA xbar-transpose instruction per m-chunk (l-part, m)
  R1[l] = sum_m E_A        (DVE free-axis reduces; shipped to host)
  A_un^T  = Qt^T @ E       (missing 1/R1[l]; host divides)
  T' = E_A^T x CbT in (m,d) layout; T = T'/r2raw (the e^{p2} factor cancels,
       so no second exp / second score pass is needed)
  Bv_un^T = T^T @ E        (missing 1/R1[l])
Host (numpy, f32): out = [C, A, C*A, C*Bv] with A = A_un/R1, Bv = Bv_un/R1.
Host also pre-packs per batch (all bf16): [Cb | Qb | QbT | CbT | p2-as-bf16x4]
so the only on-device transposes are the two E xbar transposes.
"""

import os
import threading

import numpy as np
import ml_dtypes

B, D, LC, LQ = 64, 128, 1024, 256
NCORES = 8
BPC = B // NCORES  # batches per core
BF16 = ml_dtypes.bfloat16

# packed input column offsets
OFF_CB = 0
OFF_QB = OFF_CB + LC          # 1024
OFF_QBT = OFF_QB + LQ         # 1280
OFF_CBT = OFF_QBT + LQ        # 1536
OFF_P2 = OFF_CBT + LC         # 2560
NCOLS = OFF_P2 + 4            # 2564

_lock = threading.Lock()
_cache: dict = {}


def _build_program():
    import concourse.bass as bass
    import concourse.bacc as bacc
    import concourse.mybir as mybir
    import concourse.tile as tile
    from contextlib import ExitStack

    f32 = mybir.dt.float32
    bf16 = mybir.dt.bfloat16
    MUL = mybir.AluOpType.mult
    ADD = mybir.AluOpType.add
    EXP = mybir.ActivationFunctionType.Exp
    X = mybir.AxisListType.X

    nc = bacc.Bacc("TRN2", target_bir_lowering=False)
    CQd = nc.declare_dram_parameter("CQ", [BPC, D, NCOLS], bf16, False)
    Wd = nc.declare_dram_parameter("w", [3 * D], f32, False)
    Od = nc.declare_dram_parameter("OUT", [BPC, D, 2, LC], bf16, True)
    R1d = nc.declare_dram_parameter("R1", [BPC, D, 8], f32, True)

    with ExitStack() as ctx:
        tc = ctx.enter_context(tile.TileContext(nc))
        const = ctx.enter_context(tc.tile_pool(name="const", bufs=1))
        # PSUM: big ring (128,1024)f32 tiles x3 bufs = 6 banks; tt ring 2 banks
        psb = ctx.enter_context(tc.tile_pool(name="psb", bufs=3, space="PSUM"))
        pst = ctx.enter_context(tc.tile_pool(name="pst", bufs=2, space="PSUM"))
        io = ctx.enter_context(tc.tile_pool(name="io", bufs=5))
        mid = ctx.enter_context(tc.tile_pool(name="mid", bufs=4))
        ep = ctx.enter_context(tc.tile_pool(name="ep", bufs=8))
        sm = ctx.enter_context(tc.tile_pool(name="sm", bufs=4))

        wt = const.tile([D, 3], f32)
        nc.sync.dma_start(wt[:], Wd.rearrange("(t d) -> d t", d=D))
        w1c, w3c = wt[:, 0:1], wt[:, 2:3]

        def emit_load(b):
            cq = io.tile([D, NCOLS], bf16, tag="cq", name=f"cq{b}")
            nc.sync.dma_start(cq[:], CQd[b])
            return cq

        def emit_scores(b, cq):
            """rhs1, score matmuls, exp (+row sums), E^T transposes."""
            cb = cq[:, OFF_CB : OFF_CB + LC]
            qb = cq[:, OFF_QB : OFF_QB + LQ]
            p2 = cq[:, OFF_P2 : OFF_P2 + 4].bitcast(f32)  # (128, 2) f32

            rhs1 = sm.tile([D, LQ], bf16, tag="rhs1", name=f"rhs1{b}")
            nc.gpsimd.tensor_scalar(rhs1[:], qb, w3c, w1c, op0=MUL, op1=ADD)

            r2raw = sm.tile([D, 2], f32, tag="r2raw", name=f"r2raw{b}")
            ea = mid.tile([D, 8, LQ], bf16, tag="ea", name=f"ea{b}")
            e1t = []
            for j in range(2):
                sb_ps = psb.tile([D, LC], f32, tag="psbig", name=f"sb{b}_{j}")
                lhs = rhs1[:, 128 * j : 128 * (j + 1)]
                for h in range(2):
                    nc.tensor.matmul(
                        sb_ps[:, 512 * h : 512 * (h + 1)], lhs,
                        cb[:, 512 * h : 512 * (h + 1)], start=True, stop=True,
                    )
                e = ep.tile([D, LC], bf16, tag="e1t", name=f"e{b}_{j}")
                nc.scalar.activation(
                    e[:], sb_ps[:], EXP, bias=p2[:, j : j + 1],
                    accum_out=r2raw[:, j : j + 1],
                )
                e1t.append(e)
                nc.sync.dma_start(
                    ea[:, :, 128 * j : 128 * (j + 1)], e[:], transpose=True
                )
            r2i = sm.tile([D, 2], f32, tag="r2i", name=f"r2i{b}")
            nc.vector.reciprocal(r2i[:], r2raw[:])
            return cq, ea, e1t, r2i

        def emit_mid(b, st):
            """A matmuls + eviction (ready one step after scores)."""
            cq, ea, e1t, r2i = st
            o1bv = io.tile([D, 2, LC], bf16, tag="o1bv", name=f"o1bv{b}")

            # A_un^T = Qt^T @ E  (d-part, l-free)
            a_ps = psb.tile([D, LC], f32, tag="psbig", name=f"a{b}")
            for j in range(2):
                for h in range(2):
                    nc.tensor.matmul(
                        a_ps[:, 512 * h : 512 * (h + 1)],
                        cq[:, OFF_QBT + 128 * j : OFF_QBT + 128 * (j + 1)],
                        e1t[j][:, 512 * h : 512 * (h + 1)],
                        start=(j == 0), stop=(j == 1),
                    )
            nc.scalar.copy(o1bv[:, 0, :], a_ps[:])
            return o1bv

        def emit_tail(b, st, o1bv):
            """T and Bv matmuls, R1 reduces, evictions, output DMAs
            (two steps after scores: the E^T transpose landed long ago)."""
            cq, ea, e1t, r2i = st

            # T'[m-chunk j] = sum_l E_A[l, m-chunk j] x CbT[l, :]
            tt_ps = []
            for j in range(2):
                tp = pst.tile([D, D], f32, tag="ttps", name=f"tt{b}_{j}")
                for p in range(8):
                    nc.tensor.matmul(
                        tp[:], ea[:, p, 128 * j : 128 * (j + 1)],
                        cq[:, OFF_CBT + 128 * p : OFF_CBT + 128 * (p + 1)],
                        start=(p == 0), stop=(p == 7),
                    )
                tt_ps.append(tp)

            # R1[l] = sum_m E_A[l, m]  (ready work while T matmuls run)
            r1h = sm.tile([D, 8, 2], f32, tag="r1h", name=f"r1h{b}")
            for j in range(2):
                nc.vector.tensor_reduce(
                    r1h[:, :, j], ea[:, :, 128 * j : 128 * (j + 1)], axis=X, op=ADD
                )
            r1small = sm.tile([D, 8], f32, tag="r1small", name=f"r1s{b}")
            nc.vector.tensor_tensor(
                r1small[:], r1h[:, :, 0], r1h[:, :, 1], op=ADD
            )

            # T = T'/r2raw, evicted straight from PSUM (m-part, d-free)
            tsb = sm.tile([D, LQ], bf16, tag="tsb", name=f"tsb{b}")
            for j in range(2):
                nc.vector.tensor_scalar(
                    tsb[:, 128 * j : 128 * (j + 1)], tt_ps[j][:],
                    r2i[:, j : j + 1], None, op0=MUL,
                )

            # Bv_un^T = T^T @ E  (d-part, l-free)
            bv_ps = psb.tile([D, LC], f32, tag="psbig", name=f"bv{b}")
            for j in range(2):
                for h in range(2):
                    nc.tensor.matmul(
                        bv_ps[:, 512 * h : 512 * (h + 1)],
                        tsb[:, 128 * j : 128 * (j + 1)],
                        e1t[j][:, 512 * h : 512 * (h + 1)],
                        start=(j == 0), stop=(j == 1),
                    )
            nc.vector.tensor_copy(o1bv[:, 1, :], bv_ps[:])

            nc.gpsimd.dma_start(Od[b], o1bv[:])
            nc.gpsimd.dma_start(R1d[b], r1small[:])

        # software-pipelined emission with 2-step skew: per-engine queues are
        # FIFO, so only work that is (or will momentarily be) ready may sit at
        # a queue head. scores(k+1) and A(k) fill the PE stream while the
        # E^T transpose of batch k completes; T/Bv of batch k-1 are fully
        # unblocked by the time they reach the PE.
        cqs = {0: emit_load(0)}
        st = {0: emit_scores(0, cqs[0])}
        cqs[1] = emit_load(1)
        o1bvs = {}
        for k in range(BPC + 1):
            if k + 1 < BPC:
                st[k + 1] = emit_scores(k + 1, cqs[k + 1])
            if k + 2 < BPC:
                cqs[k + 2] = emit_load(k + 2)
            if k < BPC:
                o1bvs[k] = emit_mid(k, st[k])
            if k >= 1:
                emit_tail(k - 1, st.pop(k - 1), o1bvs.pop(k - 1))

    nc.compile()
    return nc


def _get_program():
    with _lock:
        if "nc" not in _cache:
            _cache["nc"] = _build_program()
        return _cache["nc"]


def _pack_inputs(C, Q, w):
    """Per batch: [Cb | Qb | QbT | CbT | p2-as-bf16x4] as (B, D, NCOLS) bf16."""
    Cb = C.astype(BF16)
    Qb = Q.astype(BF16)
    # [m%128, m//128, d] layout for QbT, [l%128, l//128, d] for CbT
    QbT = (
        Qb.transpose(0, 2, 1).reshape(B, 2, 128, D).transpose(0, 2, 1, 3)
        .reshape(B, 128, 2 * D)
    )
    CbT = (
        Cb.transpose(0, 2, 1).reshape(B, 8, 128, D).transpose(0, 2, 1, 3)
        .reshape(B, 128, 8 * D)
    )
    p2 = np.einsum("d,bdm->bm", w[D : 2 * D], Q)
    p2c = p2.reshape(B, 2, 128).transpose(0, 2, 1).astype(np.float32)  # (B,128,2)
    p2b = np.ascontiguousarray(p2c).view(BF16)  # (B, 128, 4)
    CQ = np.empty((B, D, NCOLS), dtype=BF16)
    CQ[:, :, OFF_CB:OFF_QB] = Cb
    CQ[:, :, OFF_QB:OFF_QBT] = Qb
    CQ[:, :, OFF_QBT:OFF_CBT] = QbT
    CQ[:, :, OFF_CBT:OFF_P2] = CbT
    CQ[:, :, OFF_P2:NCOLS] = p2b
    return CQ


def kernel(C, Q, cmask, qmask, w, **_):
    # cmask/qmask are identically 1.0 for this problem; softmax masking with
    # all-ones masks is the identity, so they do not enter the computation.
    from concourse.bass_utils import run_bass_kernel_spmd

    nc = _get_program()
    C = np.ascontiguousarray(np.asarray(C), dtype=np.float32)
    Q = np.ascontiguousarray(np.asarray(Q), dtype=np.float32)
    w = np.ascontiguousarray(np.asarray(w), dtype=np.float32)
    CQ = _pack_inputs(C, Q, w)
    in_maps = [
        {
            "CQ": np.ascontiguousarray(CQ[i * BPC : (i + 1) * BPC]),
            "w": w,
        }
        for i in range(NCORES)
    ]
    res = run_bass_kernel_spmd(
        nc, in_maps, core_ids=list(range(NCORES)),
        trace=bool(int(os.environ.get("KERNEL_TRACE", "0"))),
    )
    if os.environ.get("KERNEL_RESULT_STASH") is not None:
        _cache["last_result"] = res

    out = np.empty((B, 4 * D, LC), dtype=np.float32)
    out[:, 0:D] = C
    for i in range(NCORES):
        sl = slice(i * BPC, (i + 1) * BPC)
        ob = np.asarray(res.results[i]["OUT"])  # (BPC, D, 2, LC) bf16
        r1 = np.asarray(res.results[i]["R1"])   # (BPC, D, 8) f32
        r1full = r1.transpose(0, 2, 1).reshape(BPC, LC)  # l = 128*blk + p
        inv = (1.0 / r1full)[:, None, :]
        a = ob[:, :, 0, :].astype(np.float32) * inv
        bv = ob[:, :, 1, :].astype(np.float32) * inv
        out[sl, D : 2 * D] = a
        out[sl, 2 * D : 3 * D] = C[sl] * a
        out[sl, 3 * D : 4 * D] = C[sl] * bv
    return out


# revision 17
# speedup vs baseline: 1.1386x; 1.1053x over previous
"""Context-Query (BiDAF-style) attention kernel for Trainium2, 8 NeuronCores.

Problem (per batch b of 64):
  Ct = C[b].T (Lc,D), Qt = Q[b].T (Lq,D), w = [w1,w2,w3] each (D,)
  S  = Ct@w1 + (Qt@w2).T + (Ct*w3)@Qt.T                     (Lc,Lq)
  S1 = softmax_m(S), S2 = softmax_l(S)
  A  = S1@Qt, Bv = S1@(S2.T@Ct)      (associativity: avoids Lc x Lc matrix)
  out[b] = concat([Ct, A, Ct*A, Ct*Bv], axis=1).T           (4D, Lc)

Sharding: pure data-parallel, batch 64 -> 8 cores x 8 batches.

Device computes, per batch, in bf16 with f32 accumulation:
  E = exp(S^T + p2)        (m-part, l-free; p2[m] = Qt@w2 host-computed bias)
  r2raw[m] = sum_l E[m,l]  (accum during exp eviction)
  E_A = E^T via one DMA xbar-transpose instruction per m-chunk (l-part, m)
  R1[l] = sum_m E_A        (DVE free-axis reduces; shipped to host)
  A_un^T  = Qt^T @ E       (missing 1/R1[l]; host divides)
  T' = E_A^T x CbT in (m,d) layout; T = T'/r2raw (the e^{p2} factor cancels,
       so no second exp / second score pass is needed)
  Bv_un^T = T^T @ E        (missing 1/R1[l])
Host (numpy, f32): out = [C, A, C*A, C*Bv] with A = A_un/R1, Bv = Bv_un/R1.
Host also pre-packs per batch (all bf16): [Cb | Qb | QbT | CbT | p2-as-bf16x4]
so the only on-device transposes are the two E xbar transposes.
"""

import os
import threading

import numpy as np
import ml_dtypes

B, D, LC, LQ = 64, 128, 1024, 256
NCORES = 8
BPC = B // NCORES  # batches per core
BF16 = ml_dtypes.bfloat16

# packed input column offsets
OFF_CB = 0
OFF_QB = OFF_CB + LC          # 1024
OFF_QBT = OFF_QB + LQ         # 1280
OFF_CBT = OFF_QBT + LQ        # 1536
OFF_P2 = OFF_CBT + LC         # 2560
NCOLS = OFF_P2 + 4            # 2564

_lock = threading.Lock()
_cache: dict = {}


def _build_program():
    import concourse.bass as bass
    import concourse.bacc as bacc
    import concourse.mybir as mybir
    import concourse.tile as tile
    from contextlib import ExitStack

    f32 = mybir.dt.float32
    bf16 = mybir.dt.bfloat16
    MUL = mybir.AluOpType.mult
    ADD = mybir.AluOpType.add
    EXP = mybir.ActivationFunctionType.Exp
    X = mybir.AxisListType.X

    nc = bacc.Bacc("TRN2", target_bir_lowering=False)
    CQd = nc.declare_dram_parameter("CQ", [BPC, D, NCOLS], bf16, False)
    Wd = nc.declare_dram_parameter("w", [3 * D], f32, False)
    Od = nc.declare_dram_parameter("OUT", [BPC, D, 2 * LC + 16], bf16, True)

    with ExitStack() as ctx:
        tc = ctx.enter_context(tile.TileContext(nc))
        const = ctx.enter_context(tc.tile_pool(name="const", bufs=1))
        # PSUM: big ring (128,1024)f32 tiles x3 bufs = 6 banks; tt ring 2 banks
        psb = ctx.enter_context(tc.tile_pool(name="psb", bufs=3, space="PSUM"))
        pst = ctx.enter_context(tc.tile_pool(name="pst", bufs=2, space="PSUM"))
        io = ctx.enter_context(tc.tile_pool(name="io", bufs=5))
        mid = ctx.enter_context(tc.tile_pool(name="mid", bufs=4))
        ep = ctx.enter_context(tc.tile_pool(name="ep", bufs=8))
        sm = ctx.enter_context(tc.tile_pool(name="sm", bufs=4))

        wt = const.tile([D, 3], f32)
        nc.sync.dma_start(wt[:], Wd.rearrange("(t d) -> d t", d=D))
        w1c, w3c = wt[:, 0:1], wt[:, 2:3]

        def emit_load(b):
            cq = io.tile([D, NCOLS], bf16, tag="cq", name=f"cq{b}")
            nc.sync.dma_start(cq[:], CQd[b])
            return cq

        def emit_scores(b, cq):
            """rhs1, score matmuls, exp (+row sums), E^T transposes."""
            cb = cq[:, OFF_CB : OFF_CB + LC]
            qb = cq[:, OFF_QB : OFF_QB + LQ]
            p2 = cq[:, OFF_P2 : OFF_P2 + 4].bitcast(f32)  # (128, 2) f32

            rhs1 = sm.tile([D, LQ], bf16, tag="rhs1", name=f"rhs1{b}")
            nc.gpsimd.tensor_scalar(rhs1[:], qb, w3c, w1c, op0=MUL, op1=ADD)

            r2raw = sm.tile([D, 2], f32, tag="r2raw", name=f"r2raw{b}")
            ea = mid.tile([D, 8, LQ], bf16, tag="ea", name=f"ea{b}")
            e1t = []
            for j in range(2):
                sb_ps = psb.tile([D, LC], f32, tag="psbig", name=f"sb{b}_{j}")
                lhs = rhs1[:, 128 * j : 128 * (j + 1)]
                for h in range(2):
                    nc.tensor.matmul(
                        sb_ps[:, 512 * h : 512 * (h + 1)], lhs,
                        cb[:, 512 * h : 512 * (h + 1)], start=True, stop=True,
                    )
                e = ep.tile([D, LC], bf16, tag="e1t", name=f"e{b}_{j}")
                nc.scalar.activation(
                    e[:], sb_ps[:], EXP, bias=p2[:, j : j + 1],
                    accum_out=r2raw[:, j : j + 1],
                )
                e1t.append(e)
                nc.sync.dma_start(
                    ea[:, :, 128 * j : 128 * (j + 1)], e[:], transpose=True
                )
            r2i = sm.tile([D, 2], f32, tag="r2i", name=f"r2i{b}")
            nc.vector.reciprocal(r2i[:], r2raw[:])
            return cq, ea, e1t, r2i

        def emit_mid(b, st):
            """A matmuls + eviction (ready one step after scores)."""
            cq, ea, e1t, r2i = st
            o1bv = io.tile([D, 2 * LC + 16], bf16, tag="o1bv", name=f"o1bv{b}")

            # A_un^T = Qt^T @ E  (d-part, l-free)
            a_ps = psb.tile([D, LC], f32, tag="psbig", name=f"a{b}")
            for j in range(2):
                for h in range(2):
                    nc.tensor.matmul(
                        a_ps[:, 512 * h : 512 * (h + 1)],
                        cq[:, OFF_QBT + 128 * j : OFF_QBT + 128 * (j + 1)],
                        e1t[j][:, 512 * h : 512 * (h + 1)],
                        start=(j == 0), stop=(j == 1),
                    )
            nc.scalar.copy(o1bv[:, 0:LC], a_ps[:])
            return o1bv

        def emit_tail(b, st, o1bv):
            """T and Bv matmuls, R1 reduces, evictions, output DMAs
            (two steps after scores: the E^T transpose landed long ago)."""
            cq, ea, e1t, r2i = st

            # T'[m-chunk j] = sum_l E_A[l, m-chunk j] x CbT[l, :]
            tt_ps = []
            for j in range(2):
                tp = pst.tile([D, D], f32, tag="ttps", name=f"tt{b}_{j}")
                for p in range(8):
                    nc.tensor.matmul(
                        tp[:], ea[:, p, 128 * j : 128 * (j + 1)],
                        cq[:, OFF_CBT + 128 * p : OFF_CBT + 128 * (p + 1)],
                        start=(p == 0), stop=(p == 7),
                    )
                tt_ps.append(tp)

            # R1[l] = sum_m E_A[l, m]  (ready work while T matmuls run)
            r1h = sm.tile([D, 8, 2], f32, tag="r1h", name=f"r1h{b}")
            for j in range(2):
                nc.vector.tensor_reduce(
                    r1h[:, :, j], ea[:, :, 128 * j : 128 * (j + 1)], axis=X, op=ADD
                )
            r1small = o1bv[:, 2 * LC : 2 * LC + 16].bitcast(f32)  # (128, 8)
            nc.vector.tensor_tensor(
                r1small, r1h[:, :, 0], r1h[:, :, 1], op=ADD
            )

            # T = T'/r2raw, evicted straight from PSUM (m-part, d-free)
            tsb = sm.tile([D, LQ], bf16, tag="tsb", name=f"tsb{b}")
            for j in range(2):
                nc.vector.tensor_scalar(
                    tsb[:, 128 * j : 128 * (j + 1)], tt_ps[j][:],
                    r2i[:, j : j + 1], None, op0=MUL,
                )

            # Bv_un^T = T^T @ E  (d-part, l-free)
            bv_ps = psb.tile([D, LC], f32, tag="psbig", name=f"bv{b}")
            for j in range(2):
                for h in range(2):
                    nc.tensor.matmul(
                        bv_ps[:, 512 * h : 512 * (h + 1)],
                        tsb[:, 128 * j : 128 * (j + 1)],
                        e1t[j][:, 512 * h : 512 * (h + 1)],
                        start=(j == 0), stop=(j == 1),
                    )
            nc.vector.tensor_copy(o1bv[:, LC : 2 * LC], bv_ps[:])

            nc.scalar.dma_start(Od[b], o1bv[:])

        # software-pipelined emission with 2-step skew: per-engine queues are
        # FIFO, so only work that is (or will momentarily be) ready may sit at
        # a queue head. scores(k+1) and A(k) fill the PE stream while the
        # E^T transpose of batch k completes; T/Bv of batch k-1 are fully
        # unblocked by the time they reach the PE.
        cqs = {0: emit_load(0)}
        st = {0: emit_scores(0, cqs[0])}
        cqs[1] = emit_load(1)
        o1bvs = {}
        for k in range(BPC + 1):
            if k + 1 < BPC:
                st[k + 1] = emit_scores(k + 1, cqs[k + 1])
            if k + 2 < BPC:
                cqs[k + 2] = emit_load(k + 2)
            if k < BPC:
                o1bvs[k] = emit_mid(k, st[k])
            if k >= 1:
                emit_tail(k - 1, st.pop(k - 1), o1bvs.pop(k - 1))

    nc.compile()
    return nc


def _get_program():
    with _lock:
        if "nc" not in _cache:
            _cache["nc"] = _build_program()
        return _cache["nc"]


def _pack_inputs(C, Q, w):
    """Per batch: [Cb | Qb | QbT | CbT | p2-as-bf16x4] as (B, D, NCOLS) bf16."""
    Cb = C.astype(BF16)
    Qb = Q.astype(BF16)
    # [m%128, m//128, d] layout for QbT, [l%128, l//128, d] for CbT
    QbT = (
        Qb.transpose(0, 2, 1).reshape(B, 2, 128, D).transpose(0, 2, 1, 3)
        .reshape(B, 128, 2 * D)
    )
    CbT = (
        Cb.transpose(0, 2, 1).reshape(B, 8, 128, D).transpose(0, 2, 1, 3)
        .reshape(B, 128, 8 * D)
    )
    p2 = np.einsum("d,bdm->bm", w[D : 2 * D], Q)
    p2c = p2.reshape(B, 2, 128).transpose(0, 2, 1).astype(np.float32)  # (B,128,2)
    p2b = np.ascontiguousarray(p2c).view(BF16)  # (B, 128, 4)
    CQ = np.empty((B, D, NCOLS), dtype=BF16)
    CQ[:, :, OFF_CB:OFF_QB] = Cb
    CQ[:, :, OFF_QB:OFF_QBT] = Qb
    CQ[:, :, OFF_QBT:OFF_CBT] = QbT
    CQ[:, :, OFF_CBT:OFF_P2] = CbT
    CQ[:, :, OFF_P2:NCOLS] = p2b
    return CQ


def kernel(C, Q, cmask, qmask, w, **_):
    # cmask/qmask are identically 1.0 for this problem; softmax masking with
    # all-ones masks is the identity, so they do not enter the computation.
    from concourse.bass_utils import run_bass_kernel_spmd

    nc = _get_program()
    C = np.ascontiguousarray(np.asarray(C), dtype=np.float32)
    Q = np.ascontiguousarray(np.asarray(Q), dtype=np.float32)
    w = np.ascontiguousarray(np.asarray(w), dtype=np.float32)
    CQ = _pack_inputs(C, Q, w)
    in_maps = [
        {
            "CQ": np.ascontiguousarray(CQ[i * BPC : (i + 1) * BPC]),
            "w": w,
        }
        for i in range(NCORES)
    ]
    res = run_bass_kernel_spmd(
        nc, in_maps, core_ids=list(range(NCORES)),
        trace=bool(int(os.environ.get("KERNEL_TRACE", "0"))),
    )
    if os.environ.get("KERNEL_RESULT_STASH") is not None:
        _cache["last_result"] = res

    out = np.empty((B, 4 * D, LC), dtype=np.float32)
    out[:, 0:D] = C
    for i in range(NCORES):
        sl = slice(i * BPC, (i + 1) * BPC)
        ob = np.asarray(res.results[i]["OUT"])  # (BPC, D, 2*LC+16) bf16
        r1 = ob[:, :, 2 * LC :].copy().view(np.float32)  # (BPC, D, 8)
        r1full = r1.transpose(0, 2, 1).reshape(BPC, LC)  # l = 128*blk + p
        inv = (1.0 / r1full)[:, None, :]
        a = ob[:, :, 0:LC].astype(np.float32) * inv
        bv = ob[:, :, LC : 2 * LC].astype(np.float32) * inv
        out[sl, D : 2 * D] = a
        out[sl, 2 * D : 3 * D] = C[sl] * a
        out[sl, 3 * D : 4 * D] = C[sl] * bv
    return out
